# revision 1
# baseline (speedup 1.0000x reference)
"""Trainium2 Bass kernel for the MultiHeadAttention-variant transformer block.

Math notes (derived from the module semantics):
  - The einsum 'batt,bath->bath' uses only the DIAGONAL of the softmax'd
    attention matrix, so per flat row i the attention output is
    softmax_diag_i * V[i].
  - The raw reshape (B,T,N*H)->(B,N,T,H) makes attention "blocks" couple only
    groups of 128 consecutive tokens (T/N = 1024/8 = 128), and a block's rows
    are the 8 projection chunks of those 128 tokens.
  => The whole layer is data-parallel over 128-token groups. We shard the
     4096 flattened tokens as 512 consecutive tokens per core (8 cores), with
     zero cross-core communication.

Per core (tokens ts..ts+512, xs = x_flat[ts:ts+512]):
  QT = Wq.T @ xs.T  (4096, 512)   [same for KT, VT; all bf16 matmuls]
  numer[t,n] = exp(scale * sum_h Q_n[t,h]K_n[t,h])  -- via elementwise product
     + ones-column matmul (partition reduce) -> rows on partition 0
  Attention (transposed): S^T tiles = K_n' stationary, Q moving with a 3D AP
     covering 4 heads; Z rows via ones-column matmul over exp(S^T) tiles.
  D = numer/Z on partition-0 rows; broadcast across partitions via K=1 matmul.
  hhT = VT * D[t, n(c)] ; attn = hhT.T @ Wo ; hh1 = LN(xs+attn)
  ff = relu(hh1@Wf1+bf1)@Wf2+bf2 ; out = LN(hh1+ff)
"""

import sys

sys.path.insert(0, "/opt/trn_rl_repo")

import numpy as np
import ml_dtypes

import concourse.bass as bass
import concourse.mybir as mybir
import concourse.tile as tile
from concourse import bacc, bass_utils

F32 = mybir.dt.float32
BF16 = mybir.dt.bfloat16
F8 = mybir.dt.float8e4
AF = mybir.ActivationFunctionType
ALU = mybir.AluOpType
AX = mybir.AxisListType

H = 512
NH = 8
B = 4
T = 1024
TOK = B * T
NCORES = 8
TPC = TOK // NCORES  # 512 tokens per core
NBLK = TPC // 128  # 4 attention blocks per core
SCALE = float(1.0 / np.sqrt(H))
LN_EPS = 1e-5

_BF = ml_dtypes.bfloat16
_F8 = ml_dtypes.float8_e4m3
HSC = 512.0  # hhT fp8 dynamic-range rescale


def _emit(nc, tc, d):
    """Emit the per-core program. d: dict of DRAM APs."""
    P = tc.alloc_tile_pool(name="persist", bufs=1)
    PW = tc.alloc_tile_pool(name="wpool", bufs=12)
    SCR = tc.alloc_tile_pool(name="scr", bufs=4)
    ST = tc.alloc_tile_pool(name="stats", bufs=4)
    BIG = tc.alloc_tile_pool(name="bigp", bufs=2)
    PSA = tc.alloc_tile_pool(name="psa", bufs=4, space="PSUM")
    PSS = tc.alloc_tile_pool(name="pss", bufs=2, space="PSUM")
    PSZ = tc.alloc_tile_pool(name="psz", bufs=2, space="PSUM")

    # ---- persistent tiles
    xT = P.tile([128, 4 * 512], F8, name="xT")  # x^T, 4 h-chunks
    identf = P.tile([128, 128], F32, name="identf")
    identb = P.tile([128, 128], BF16, name="identb")
    ones_c = P.tile([128, 1], BF16, name="ones_c")  # lhsT for partition sums
    ones_b = P.tile([1, 128], BF16, name="ones_b")  # lhsT for broadcasts
    vrow = P.tile([1, 4 * 512], BF16, name="vrow")  # g1,vsum,g2,b2 rows
    bf1c = P.tile([128, 16], F32, name="bf1c")  # bf1 column-major
    gb = P.tile([128, 4 * 512], F32, name="gb")  # bcast g1,vsum,g2,b2
    Bc = P.tile([128, 8 * 512], BF16, name="Bc")  # diag-softmax bcast per n
    nrow = P.tile([1, 8 * 512], BF16, name="nrow")  # exp(diag*scale) rows
    DTr = P.tile([1, 8 * 512], BF16, name="DTr")  # D rows (n-major, t)

    # ---- input DMAs
    for hc in range(4):
        nc.sync.dma_start(xT[:, hc * 512:(hc + 1) * 512],
                          d["xT"][hc * 128:(hc + 1) * 128, :])
    nc.vector.memset(ones_c[:], 1.0)
    nc.vector.memset(ones_b[:], 1.0)
    epsc = P.tile([128, 1], F32, name="epsc")
    nc.vector.memset(epsc[:], LN_EPS)

    QT = BIG.tile([128, 32 * 512], F8, name="QT", tag="big")
    KT = BIG.tile([128, 32 * 512], F8, name="KT", tag="big")

    # ---- projections (fp8 DoubleRow): dst^T = W^T @ x^T, h_in chunk pairs
    # packed along the free axis. Weight DRAM is pre-packed tile-major:
    # tile (pair, q) rows, [p, j*1024+m] = W[(2*pair+j)*128+p, q*1024+m].
    xTp = [xT[:, pp * 1024:(pp + 1) * 1024].rearrange("p (j t) -> p j t", j=2)
           for pp in range(2)]

    def proj(wname, evac):
        wsrc = d[wname].rearrange("(t p) f -> t p f", p=128)
        wt = {}
        for q in range(4):
            for pp in range(2):
                w = PW.tile([128, 2048], F8, name=f"w_{wname}{q}{pp}", tag="w")
                nc.sync.dma_start(w[:], wsrc[pp * 4 + q])
                wt[(q, pp)] = w
        for m in range(32):
            q, mq = m // 8, m % 8
            ps = PSA.tile([128, 512], F32, name="ps_proj", tag="acc")
            for pp in range(2):
                lhsT = wt[(q, pp)].rearrange(
                    "p (j m) -> p j m", j=2)[:, :, mq * 128:(mq + 1) * 128]
                nc.tensor.matmul(
                    ps[:], lhsT=lhsT, rhs=xTp[pp],
                    start=(pp == 0), stop=(pp == 1),
                    perf_mode=mybir.MatmulPerfMode.DoubleRow)
            evac(m, ps)

    def evac_alt(dst):
        def f(m, ps):
            sl = dst[:, m * 512:(m + 1) * 512]
            if m % 2 == 0:
                nc.vector.tensor_copy(sl, ps[:])
            else:
                nc.scalar.copy(sl, ps[:])
        return f

    proj("wq", evac_alt(QT))
    proj("wk", evac_alt(KT))

    # ---- attention: S^T tiles (t' partitions, (n,t) moving), Z row sums
    nc.sync.dma_start(identf[:], d["ident"][:])
    nc.vector.tensor_copy(identb[:], identf[:])
    QT3 = QT.rearrange("p (m t) -> p m t", t=512)
    KT3 = KT.rearrange("p (m t) -> p m t", t=512)
    # QT6[p, gg, n, c, t]: chunk m = 16*gg + 4*n + c, c = 2*pp + j
    QT6 = QT.rearrange("p (gg n c t) -> p gg n c t", gg=2, n=4, c=4)

    def z_chain(a, exs):
        def emit():
            for g in range(2):
                zps = PSZ.tile([1, 512], F32, name="zps", tag="zr")
                for n2 in range(NH):
                    nc.tensor.matmul(zps[:], lhsT=ones_c[:],
                                     rhs=exs[(n2, g)][:],
                                     start=(n2 == 0), stop=(n2 == NH - 1))
                zs_ = SCR.tile([1, 512], F32, name="zs_", tag="zs", bufs=3)
                nc.scalar.mul(zs_[:], zps[:], 1.0 / HSC)
                zr_ = SCR.tile([1, 512], F32, name="zr_", tag="zs", bufs=3)
                nc.vector.reciprocal(zr_[:], zs_[:])
                for j in range(4):
                    n = g * 4 + j
                    nc.vector.tensor_mul(
                        DTr[0:1, n * 512 + a * 128:n * 512 + a * 128 + 128],
                        nrow[0:1, n * 512 + a * 128:n * 512 + a * 128 + 128],
                        zr_[0:1, j * 128:(j + 1) * 128])
        return emit

    pending_z = None
    for a in range(NBLK):
        exs = {}
        cnt = 0
        for n2 in range(NH):  # stationary K head
            for g in range(2):  # moving Q 4-head group
                ps = PSS.tile([128, 512], F32, name="ps_s", tag="S")
                for pp in range(2):
                    base = 4 * n2 + 2 * pp
                    lhsT = KT3[:, base:base + 2, a * 128:(a + 1) * 128]
                    # [p][n][j][t] -> [p][j][n][t]
                    rhs = QT6[:, g, :, 2 * pp:2 * pp + 2,
                              a * 128:(a + 1) * 128].transpose([0, 2, 1, 3])
                    nc.tensor.matmul(ps[:], lhsT=lhsT, rhs=rhs,
                                     start=(pp == 0), stop=(pp == 1),
                                     perf_mode=mybir.MatmulPerfMode.DoubleRow)
                ex = SCR.tile([128, 512], BF16, name="ex", tag="ex", bufs=24)
                nc.scalar.activation(ex[:], ps[:], AF.Exp, scale=SCALE)
                exs[(n2, g)] = ex
                if g == n2 // 4:
                    j = n2 % 4
                    msk = SCR.tile([128, 128], BF16, name="msk", tag="msk",
                                   bufs=3)
                    nc.vector.tensor_mul(msk[:], ex[:, j * 128:(j + 1) * 128],
                                         identb[:])
                    nps = PSZ.tile([1, 128], F32, name="nps", tag="zr")
                    nc.tensor.matmul(nps[:], lhsT=ones_c[:], rhs=msk[:],
                                     start=True, stop=True)
                    nc.vector.tensor_copy(
                        nrow[0:1, n2 * 512 + a * 128:n2 * 512 + a * 128 + 128],
                        nps[:])
                cnt += 1
                if cnt == 4 and pending_z is not None:
                    pending_z()
                    pending_z = None
        pending_z = z_chain(a, exs)
    pending_z()

    # ---- late constants: identity, gamma/beta rows, bf1 columns
    for i in range(4):
        nc.sync.dma_start(vrow[0:1, i * 512:(i + 1) * 512], d["vecs"][i:i + 1, :])
    nc.sync.dma_start(bf1c[:], d["bf1"].rearrange("(m p) -> p m", p=128))
    for i in range(4):
        psg = PSA.tile([128, 512], F32, name="psg", tag="acc")
        nc.tensor.matmul(psg[:], lhsT=ones_b[:],
                         rhs=vrow[0:1, i * 512:(i + 1) * 512],
                         start=True, stop=True, tile_position=(0, 0))
        nc.scalar.copy(gb[:, i * 512:(i + 1) * 512], psg[:])

    # ---- broadcast D rows across partitions: Bc_n[p, t] = D[t, n]
    for n in range(NH):
        psb = PSA.tile([128, 512], F32, name="psb", tag="acc")
        nc.tensor.matmul(psb[:], lhsT=ones_b[:],
                         rhs=DTr[0:1, n * 512:(n + 1) * 512],
                         start=True, stop=True, tile_position=(0, 0))
        nc.vector.tensor_copy(Bc[:, n * 512:(n + 1) * 512], psb[:])

    # ---- V projection fused with diag-softmax scaling -> hhT
    hhT = BIG.tile([128, 32 * 512], F8, name="hhT", tag="big")
    proj("wv", lambda m, ps: nc.vector.tensor_mul(
        hhT[:, m * 512:(m + 1) * 512], ps[:],
        Bc[:, (m // 4) * 512:(m // 4 + 1) * 512]))

    # ---- layernorm helpers (per 128-token tile, free dim = 512 hidden)
    def ln_core(v_ap, out_ap):
        nmu = ST.tile([128, 1], F32, name="nmu", tag="nmu")
        nc.vector.reduce_sum(nmu[:], v_ap, axis=AX.X)
        nc.vector.tensor_scalar_mul(nmu[:], nmu[:], -1.0 / H)
        ssq = ST.tile([128, 1], F32, name="ssq", tag="ssq")
        junkf = SCR.tile([128, 512], BF16, name="junkf", tag="junkf", bufs=2)
        nc.scalar.activation(junkf[:], v_ap, AF.Square, bias=nmu[:],
                             accum_out=ssq[:])
        sd = ST.tile([128, 1], F32, name="sd", tag="sd")
        nc.scalar.activation(sd[:], ssq[:], AF.Sqrt, scale=1.0 / H,
                             bias=epsc[:])
        rs = ST.tile([128, 1], F32, name="rs", tag="rs")
        nc.vector.reciprocal(rs[:], sd[:])
        nc.vector.tensor_scalar(out_ap, v_ap, nmu[:], rs[:],
                                op0=ALU.add, op1=ALU.mult)

    def ln(v_ap, gofs, bofs, out_ap):
        ln_core(v_ap, out_ap)
        nc.vector.tensor_mul(out_ap, out_ap, gb[:, gofs * 512:(gofs + 1) * 512])
        nc.vector.tensor_add(out_ap, out_ap, gb[:, bofs * 512:(bofs + 1) * 512])

    # ---- output projection + residual + LN1
    xcr = BIG.tile([128, 4 * 512], F32, name="xcr", tag="med", bufs=1)
    wo_t = []
    wosrc = d["wo"].rearrange("(t p) f -> t p f", p=128)
    for i in range(16):
        w = PW.tile([128, 1024], F8, name=f"w_wo{i}", tag="w")
        nc.sync.dma_start(w[:], wosrc[i])
        wo_t.append(w)
    hhT3 = hhT.rearrange("p (m t) -> p m t", t=512)
    ps_o = [PSA.tile([128, 512], F32, name=f"ps_o{mt}", tag="acc")
            for mt in range(4)]
    for i in range(16):
        for mt in range(4):
            lhsT = hhT3[:, 2 * i:2 * i + 2, mt * 128:(mt + 1) * 128]
            rhs = wo_t[i].rearrange("p (j h) -> p j h", j=2)
            nc.tensor.matmul(
                ps_o[mt][:], lhsT=lhsT, rhs=rhs,
                start=(i == 0), stop=(i == 15), skip_group_check=True,
                perf_mode=mybir.MatmulPerfMode.DoubleRow)
    for mt in range(4):
        xr = SCR.tile([128, 512], F32, name="xr", tag="xr")
        nc.sync.dma_start(xr[:], d["xr"][mt * 128:(mt + 1) * 128, :])
        v1 = SCR.tile([128, 512], F32, name="v1", tag="xr")
        nc.vector.scalar_tensor_tensor(
            out=v1[:], in0=ps_o[mt][:], scalar=1.0 / HSC, in1=xr[:],
            op0=ALU.mult, op1=ALU.add)
        ln_core(v1[:], xcr[:, mt * 512:(mt + 1) * 512])

    # ---- transpose xcr -> hh1T (bf16) for the FFN (g1/b1 folded into Wf1);
    # hh1r = xcr*g1 + (beta1+bf2) computed off the critical path
    hh1T = BIG.tile([128, 4 * 512], BF16, name="hh1T", tag="med2", bufs=1)
    hh1r = BIG.tile([128, 4 * 512], F32, name="hh1r", tag="med3", bufs=1)
    for mt in range(4):
        for j in range(4):
            tp = PSZ.tile([128, 128], F32, name="tp_h", tag="zr")
            nc.tensor.transpose(
                tp[:], xcr[:, mt * 512 + j * 128:mt * 512 + j * 128 + 128],
                identf[:])
            nc.vector.tensor_copy(
                hh1T[:, j * 512 + mt * 128:j * 512 + mt * 128 + 128], tp[:])
    for mt in range(4):
        sl = slice(mt * 512, (mt + 1) * 512)
        nc.vector.tensor_mul(hh1r[:, sl], xcr[:, sl], gb[:, 0:512])
        nc.vector.tensor_add(hh1r[:, sl], hh1r[:, sl], gb[:, 512:1024])

    # ---- FFN1: a1T = relu(Wf1^T @ hh1T + bf1)
    a1T = BIG.tile([128, 16 * 512], BF16, name="a1T", tag="big")
    wf1_t = {}
    for fq in range(2):
        for hc in range(4):
            w = PW.tile([128, 1024], BF16, name=f"w_wf1{fq}{hc}", tag="w")
            nc.sync.dma_start(
                w[:], d["wf1"][hc * 128:(hc + 1) * 128,
                               fq * 1024:(fq + 1) * 1024])
            wf1_t[(fq, hc)] = w
    for mf in range(16):
        ps = PSA.tile([128, 512], F32, name="ps_f1", tag="acc")
        for hc in range(4):
            nc.tensor.matmul(
                ps[:],
                lhsT=wf1_t[(mf // 8, hc)][:, (mf % 8) * 128:
                                           (mf % 8) * 128 + 128],
                rhs=hh1T[:, hc * 512:(hc + 1) * 512],
                start=(hc == 0), stop=(hc == 3))
        nc.scalar.activation(a1T[:, mf * 512:(mf + 1) * 512], ps[:], AF.Relu,
                             bias=bf1c[:, mf:mf + 1])

    # ---- FFN2 + residual + bf2 + LN2 -> out
    wf2_t = []
    for i in range(8):
        w = PW.tile([128, 1024], BF16, name=f"w_wf2{i}", tag="w")
        nc.sync.dma_start(
            w.rearrange("p (c h) -> p c h", h=512),
            d["wf2"].rearrange("(c p) h -> p c h", p=128)[:, 2 * i:2 * i + 2, :])
        wf2_t.append(w)
    for mt in range(4):
        ps = PSA.tile([128, 512], F32, name="ps_f2", tag="acc")
        for fc in range(16):
            nc.tensor.matmul(
                ps[:], lhsT=a1T[:, fc * 512 + mt * 128:fc * 512 + mt * 128 + 128],
                rhs=wf2_t[fc // 2][:, (fc % 2) * 512:(fc % 2 + 1) * 512],
                start=(fc == 0), stop=(fc == 15))
        s2 = SCR.tile([128, 512], F32, name="s2", tag="xr")
        nc.vector.tensor_add(s2[:], ps[:], hh1r[:, mt * 512:(mt + 1) * 512])
        outt = SCR.tile([128, 512], F32, name="outt", tag="xr")
        ln(s2[:], 2, 3, outt[:])
        nc.sync.dma_start(d["out"][mt * 128:(mt + 1) * 128, :], outt[:])

    for pool in (PSZ, PSS, PSA, BIG, ST, SCR, PW, P):
        pool.release()


def build(loop_n=None):
    nc = bacc.Bacc("TRN2", target_bir_lowering=False)
    d = {
        "xT": nc.dram_tensor("xT", (TPC, H), F8, kind="ExternalInput").ap(),
        "xr": nc.dram_tensor("xr", (TPC, H), F32, kind="ExternalInput").ap(),
        "wq": nc.dram_tensor("wq", (1024, 2048), F8, kind="ExternalInput").ap(),
        "wk": nc.dram_tensor("wk", (1024, 2048), F8, kind="ExternalInput").ap(),
        "wv": nc.dram_tensor("wv", (1024, 2048), F8, kind="ExternalInput").ap(),
        "wo": nc.dram_tensor("wo", (2048, 1024), F8, kind="ExternalInput").ap(),
        "wf1": nc.dram_tensor("wf1", (H, 4 * H), BF16, kind="ExternalInput").ap(),
        "wf2": nc.dram_tensor("wf2", (4 * H, H), BF16, kind="ExternalInput").ap(),
        "bf1": nc.dram_tensor("bf1", (4 * H,), F32, kind="ExternalInput").ap(),
        "vecs": nc.dram_tensor("vecs", (4, H), BF16, kind="ExternalInput").ap(),
        "ident": nc.dram_tensor("ident", (128, 128), F32,
                                kind="ExternalInput").ap(),
        "out": nc.dram_tensor("out", (TPC, H), F32, kind="ExternalOutput").ap(),
    }
    with tile.TileContext(nc) as tc:
        if loop_n is None:
            _emit(nc, tc, d)
        else:
            with tc.For_i(0, loop_n, 1):
                _emit(nc, tc, d)
    nc.finalize()
    return nc


def _pack_w(W):
    # tile-major fp8 packing for DoubleRow projections:
    # tile (pair, q): [p, j*1024+m] = W[(2*pair+j)*128+p, q*1024+m]
    W5 = np.asarray(W, np.float32).reshape(2, 2, 128, 4, 1024)
    return np.ascontiguousarray(
        W5.transpose(0, 3, 2, 1, 4).reshape(8 * 128, 2048)).astype(_F8)


def _pack_wo(W):
    # tile i: [p, j*512+h] = Wo[(2*i+j)*128+p, h]
    W4 = np.asarray(W, np.float32).reshape(16, 2, 128, 512)
    return np.ascontiguousarray(
        W4.transpose(0, 2, 1, 3).reshape(16 * 128, 1024)).astype(_F8)


def make_in_maps(inputs):
    xf = np.ascontiguousarray(
        np.asarray(inputs["x"], np.float32).reshape(TOK, H))
    shared = {
        "wq": _pack_w(inputs["Wq"]),
        "wk": _pack_w(inputs["Wk"]),
        "wv": _pack_w(inputs["Wv"]),
        "wo": _pack_wo(inputs["Wo"]),
        "wf1": (np.asarray(inputs["g1"], np.float32)[:, None]
                * np.asarray(inputs["Wf1"], np.float32)).astype(_BF),
        "wf2": np.asarray(inputs["Wf2"], np.float32).astype(_BF),
        "bf1": (np.asarray(inputs["bf1"], np.float32)
                + np.asarray(inputs["beta1"], np.float32)
                @ np.asarray(inputs["Wf1"], np.float32)),
        "vecs": np.ascontiguousarray(np.stack([
            np.asarray(inputs["g1"], np.float32),
            np.asarray(inputs["beta1"], np.float32)
            + np.asarray(inputs["bf2"], np.float32),
            np.asarray(inputs["g2"], np.float32),
            np.asarray(inputs["beta2"], np.float32)]).astype(_BF)),
        "ident": np.eye(128, dtype=np.float32),
    }
    in_maps = []
    for c in range(NCORES):
        xs = xf[c * TPC:(c + 1) * TPC]
        m = dict(shared)
        m["xT"] = np.ascontiguousarray(xs.T).astype(_F8)
        m["xr"] = np.ascontiguousarray(xs)
        in_maps.append(m)
    return in_maps


_nc_cache = None


def _get_nc():
    global _nc_cache
    if _nc_cache is None:
        _nc_cache = build()
    return _nc_cache


def kernel(**inputs):
    nc = _get_nc()
    in_maps = make_in_maps(inputs)
    res = bass_utils.run_bass_kernel_spmd(nc, in_maps,
                                          core_ids=list(range(NCORES)))
    out = np.concatenate([r["out"] for r in res.results], axis=0)
    return out.reshape(B, T, H)


if __name__ == "__main__":
    nc = build()
    n_inst = sum(len(bb.instructions) for bb in nc.main_func.blocks)
    print("built OK; instructions:", n_inst)

